# revision 18
# baseline (speedup 1.0000x reference)
"""Trainium2 Bass kernel for nn_Decoder (attention decoder step + LSTM cell + vocab head).

Sharding: data-parallel over batch N=256 across 8 cores for the attention+LSTM
part; tensor-parallel over the 32k vocab for the output Linear, with an
on-device AllGather of h_new between the two phases. Host does only slicing /
weight layout transposes and final concatenation.
"""

import os
import sys

sys.path.insert(0, "/opt/trn_rl_repo")

import numpy as np

import concourse.bacc as bacc
import concourse.bass as bass
import concourse.tile as tile
from concourse import mybir
from concourse.bass_utils import run_bass_kernel_spmd
from concourse.masks import make_identity

NCORES = 8
V, E, H, O, S, N = 32000, 256, 512, 32000, 128, 256
NC = N // NCORES          # 32 batch rows per core
OC = O // NCORES          # 4000 vocab entries per core
TWOH = 2 * H              # 1024
FOURH = 4 * H             # 2048
KF = TWOH + E + H         # 1792 gates contraction (context | emb | h)
NKT = KF // 128           # 14 k-tiles
ECH = TWOH // 128         # 8 e-chunks of encoder features
NB = 8                    # batch-group size for softmax/context pipelining
VW = 500                  # vocab tile width for the output matmul
NVH = 2                   # vocab halves (streamed W_out)
VPH = OC // NVH           # 2000 vocab cols per half
NVT = VPH // VW           # 4 vocab tiles per half

f32 = mybir.dt.float32
f32r = mybir.dt.float32r
bf16 = mybir.dt.bfloat16
i32 = mybir.dt.int32
AF = mybir.ActivationFunctionType
ALU = mybir.AluOpType


def _r(ap):
    """View a fp32 AP as float32r for full-rate PE matmuls."""
    return ap.bitcast(f32r)


def build_program():
    nc = bacc.Bacc(
        "TRN2", target_bir_lowering=False, debug=False, num_devices=NCORES
    )

    # ---- per-core external I/O ----
    x_t = nc.dram_tensor("x_idx", [NC, 1], i32, kind="ExternalInput")
    enc_t = nc.dram_tensor("enc", [NC, S, TWOH], f32, kind="ExternalInput")
    hT_t = nc.dram_tensor("h_T", [H, NC], f32, kind="ExternalInput")
    cell_t = nc.dram_tensor("cell", [NC, H], f32, kind="ExternalInput")
    emb_t = nc.dram_tensor("emb", [V, E], f32, kind="ExternalInput")
    we_t = nc.dram_tensor("w_energy", [1, 3 * H], f32, kind="ExternalInput")
    be_t = nc.dram_tensor("b_energy", [1, 1], f32, kind="ExternalInput")
    wg_t = nc.dram_tensor("w_gates_T", [KF, FOURH], f32, kind="ExternalInput")
    bih_t = nc.dram_tensor("b_ih", [1, FOURH], f32, kind="ExternalInput")
    bhh_t = nc.dram_tensor("b_hh", [1, FOURH], f32, kind="ExternalInput")
    wo_t = nc.dram_tensor("w_out_T", [H, OC], f32, kind="ExternalInput")
    bo_t = nc.dram_tensor("b_out", [1, OC], f32, kind="ExternalInput")

    logits_t = nc.dram_tensor("logits", [N, OC], f32, kind="ExternalOutput")
    hnew_t = nc.dram_tensor("h_new", [NC, H], f32, kind="ExternalOutput")
    cnew_t = nc.dram_tensor("c_new", [NC, H], f32, kind="ExternalOutput")

    # collective bounce buffers (DRAM)
    hT_bounce = nc.dram_tensor("hT_bounce", [H, NC], f32)
    hT_all = nc.dram_tensor("hT_all", [NCORES, H, NC], f32, addr_space="Shared")

    with tile.TileContext(nc) as tc:
        _build_body(
            nc, tc,
            x_t, enc_t, hT_t, cell_t, emb_t, we_t, be_t, wg_t, bih_t, bhh_t,
            wo_t, bo_t, logits_t, hnew_t, cnew_t, hT_bounce, hT_all,
        )
    nc.compile()
    return nc


def _build_body(
    nc, tc,
    x_t, enc_t, hT_t, cell_t, emb_t, we_t, be_t, wg_t, bih_t, bhh_t,
    wo_t, bo_t, logits_t, hnew_t, cnew_t, hT_bounce, hT_all,
):
    from contextlib import ExitStack

    ctx = ExitStack()
    with ctx:
        consts = ctx.enter_context(tc.tile_pool(name="consts", bufs=1))
        enc_pool = ctx.enter_context(tc.tile_pool(name="encp", bufs=1))
        small = ctx.enter_context(tc.tile_pool(name="small", bufs=2))
        prods = ctx.enter_context(tc.tile_pool(name="prods", bufs=2))
        wt_pool = ctx.enter_context(tc.tile_pool(name="wtp", bufs=2))
        wo_pool = ctx.enter_context(tc.tile_pool(name="wop", bufs=2))
        hA_pool = ctx.enter_context(tc.tile_pool(name="hAp", bufs=1))
        out_pool = ctx.enter_context(tc.tile_pool(name="outp", bufs=3))

        # ---------------- constants / small loads ----------------
        identity = consts.tile([128, 128], f32)
        make_identity(nc, identity)

        # W_energy enc part as bf16, replicated to all 128 partitions
        we_e_bf = consts.tile([S, TWOH], bf16)
        nc.gpsimd.dma_start(
            out=we_e_bf, in_=we_t[0:1, H : 3 * H].to_broadcast([S, TWOH])
        )
        # W_energy h part as [128, 4] (k-chunk layout)
        we_h = consts.tile([128, 4], f32r)
        nc.gpsimd.dma_start(
            out=we_h, in_=we_t.ap().rearrange("o (c k) -> (o k) c", k=128)[:, 0:4]
        )
        be_bc = consts.tile([128, 1], f32)
        nc.sync.dma_start(out=be_bc, in_=be_t[0:1, 0:1].to_broadcast([128, 1]))

        # h^T [512, 32] -> SBUF [128, 4, 32]
        hT_sb = consts.tile([128, 4, NC], f32r)
        nc.gpsimd.dma_start(
            out=hT_sb, in_=hT_t.ap().rearrange("(c k) n -> k c n", k=128)
        )
        cell_sb = consts.tile([NC, H], f32)
        nc.sync.dma_start(out=cell_sb, in_=cell_t[:])

        # embedding gather
        x_sb = consts.tile([NC, 1], i32)
        nc.sync.dma_start(out=x_sb, in_=x_t[:])
        emb_sb = consts.tile([NC, E], f32)
        nc.gpsimd.indirect_dma_start(
            out=emb_sb[:],
            out_offset=None,
            in_=emb_t[:],
            in_offset=bass.IndirectOffsetOnAxis(ap=x_sb[:, 0:1], axis=0),
        )

        # enc in SBUF as [s=128, n=32, e=1024] bf16 (cast during DMA)
        enc_bf = enc_pool.tile([S, NC, TWOH], bf16)
        enc_src = enc_t.ap().rearrange("n s e -> s n e")
        DMA_GRP = 4  # n rows per DMA
        for g in range(NC // DMA_GRP):
            sl = slice(g * DMA_GRP, (g + 1) * DMA_GRP)
            nc.gpsimd.dma_start(out=enc_bf[:, sl, :], in_=enc_src[:, sl, :])

        eterm = consts.tile([S, NC], f32)
        ctxT_sb = consts.tile([128, ECH, NC], f32r)
        embT_sb = consts.tile([128, 2, NC], f32r)
        hterm_bc = consts.tile([S, NC], f32)

        with (
            tc.tile_pool(name="ps_attn", bufs=3, space="PSUM") as ps_attn,
            tc.tile_pool(name="ps_ctx", bufs=1, space="PSUM") as ps_ctx,
        ):
            # emb^T chunks [128, 2, 32]
            for c in range(2):
                ps = ps_attn.tile([128, 128], f32, tag="tr")
                nc.tensor.transpose(
                    out=ps[:, 0:NC],
                    in_=emb_sb[:, 128 * c : 128 * (c + 1)],
                    identity=identity[0:NC, 0:NC],
                )
                nc.vector.tensor_copy(out=embT_sb[:, c, :], in_=ps[:, 0:NC])

            # hterm[n] = h[n] . We_h + b_energy, replicated on all partitions
            # via a free-broadcast stationary operand -> out [128, NC]
            hterm_ps = ps_attn.tile([128, 128], f32, tag="tr")
            for c in range(4):
                nc.tensor.matmul(
                    out=hterm_ps[:, 0:NC],
                    lhsT=we_h[:, c : c + 1].to_broadcast([128, 128]),
                    rhs=hT_sb[:, c, :],
                    start=(c == 0),
                    stop=(c == 3),
                )
            nc.vector.tensor_copy(out=hterm_bc, in_=hterm_ps[:, 0:NC])
            nc.vector.tensor_scalar_add(
                out=hterm_bc, in0=hterm_bc, scalar1=be_bc[:, 0:1]
            )

            # ---------------- encoder attention ----------------
            ctx_ps = ps_ctx.tile([128, ECH, NC], f32)

            for bg in range(NC // NB):  # 4 batch groups of NB=8
                nsl = slice(bg * NB, (bg + 1) * NB)
                # energy e-term per n: mul (DVE TT bf16 2x) then reduce via
                # tensor_scalar accumulate (DVE 4x bf16)
                for n in range(bg * NB, (bg + 1) * NB):
                    prod = prods.tile([S, TWOH], bf16, tag="prod")
                    nc.vector.tensor_tensor(
                        out=prod, in0=enc_bf[:, n, :], in1=we_e_bf, op=ALU.mult
                    )
                    dump = prods.tile([S, TWOH], bf16, tag="dump")
                    nc.vector.tensor_scalar(
                        out=dump,
                        in0=prod,
                        scalar1=1.0,
                        scalar2=0.0,
                        op0=ALU.mult,
                        op1=ALU.add,
                        accum_out=eterm[:, n : n + 1],
                    )
                # energy = relu(eterm + hterm) in [s, nb] layout
                en_sn = small.tile([S, NB], f32, tag="en_sn")
                nc.vector.tensor_tensor(
                    out=en_sn,
                    in0=eterm[:, nsl],
                    in1=hterm_bc[:, nsl],
                    op=ALU.add,
                )
                nc.vector.tensor_scalar_max(out=en_sn, in0=en_sn, scalar1=0.0)
                # transpose to [nb, s]
                tr_ps = ps_attn.tile([128, 128], f32, tag="tr")
                nc.tensor.transpose(out=tr_ps[0:NB, :], in_=en_sn, identity=identity)
                en_ns = small.tile([NB, S], f32, tag="en_ns")
                nc.vector.tensor_copy(out=en_ns, in_=tr_ps[0:NB, :])
                # softmax over s
                negmax = small.tile([NB, 1], f32, tag="negmax")
                nc.vector.tensor_reduce(
                    out=negmax, in_=en_ns, axis=mybir.AxisListType.X, op=ALU.max,
                    negate=True,
                )
                att_e = small.tile([NB, S], f32, tag="att_e")
                nc.scalar.activation(
                    out=att_e, in_=en_ns, func=AF.Exp, bias=negmax, scale=1.0
                )
                ssum = small.tile([NB, 1], f32, tag="ssum")
                nc.vector.tensor_reduce(
                    out=ssum, in_=att_e, axis=mybir.AxisListType.X, op=ALU.add
                )
                rinv = small.tile([NB, 1], f32, tag="rinv")
                nc.vector.reciprocal(out=rinv, in_=ssum)
                att = small.tile([NB, S], f32, tag="att")
                nc.vector.tensor_scalar_mul(out=att, in0=att_e, scalar1=rinv)
                # att^T -> [s, nb] bf16
                attT_ps = ps_attn.tile([128, 128], f32, tag="tr")
                nc.tensor.transpose(
                    out=attT_ps[:, 0:NB], in_=att, identity=identity[0:NB, 0:NB]
                )
                attT_bf = small.tile([S, NB], bf16, tag="attT_bf")
                nc.vector.tensor_copy(out=attT_bf, in_=attT_ps[:, 0:NB])
                # context^T chunks: ctx_ps[:, c, n] = enc[n,:,128c:...]^T @ att[n]
                for ni in range(NB):
                    n = bg * NB + ni
                    for c in range(ECH):
                        nc.tensor.matmul(
                            out=ctx_ps[:, c, n : n + 1],
                            lhsT=enc_bf[:, n, 128 * c : 128 * (c + 1)],
                            rhs=attT_bf[:, ni : ni + 1],
                            start=True,
                            stop=True,
                        )

            nc.vector.tensor_copy(out=ctxT_sb, in_=ctx_ps)

        # ---------------- LSTM gates ----------------
        # bias injection k-tile (K=2): row0 = b_ih, row1 = b_hh, ones lhsT
        brow_bf = consts.tile([2, FOURH], bf16)
        nc.gpsimd.dma_start(out=brow_bf[0:1, :], in_=bih_t[:])
        nc.gpsimd.dma_start(out=brow_bf[1:2, :], in_=bhh_t[:])
        ones2_bf = consts.tile([2, NC], bf16)
        nc.vector.memset(ones2_bf, 1.0)

        lhs_tiles = (
            [ctxT_sb[:, c, :] for c in range(ECH)]
            + [embT_sb[:, c, :] for c in range(2)]
            + [hT_sb[:, c, :] for c in range(4)]
        )
        wg_tiles = []
        for ki in range(NKT):
            wk = wt_pool.tile([128, FOURH], f32r, tag="wt")
            nc.gpsimd.dma_start(out=wk, in_=wg_t[128 * ki : 128 * (ki + 1), :])
            wg_tiles.append(wk)

        acts = consts.tile([NC, FOURH], f32)
        hnew_sb = consts.tile([NC, H], f32)
        cnew_sb = consts.tile([NC, H], f32)

        with tc.tile_pool(name="ps_gates", bufs=1, space="PSUM") as ps_gates:
            gates_ps = ps_gates.tile([NC, FOURH], f32)
            for ki in range(NKT):
                for j in range(FOURH // 512):
                    jsl = slice(512 * j, 512 * (j + 1))
                    nc.tensor.matmul(
                        out=gates_ps[:, jsl],
                        lhsT=lhs_tiles[ki],
                        rhs=wg_tiles[ki][:, jsl],
                        start=(ki == 0),
                        stop=False,
                    )
            for j in range(FOURH // 512):
                jsl = slice(512 * j, 512 * (j + 1))
                nc.tensor.matmul(
                    out=gates_ps[:, jsl],
                    lhsT=ones2_bf,
                    rhs=brow_bf[:, jsl],
                    start=False,
                    stop=True,
                )

            # ---------------- LSTM cell ----------------
            nc.scalar.activation(
                out=acts[:, 0:TWOH], in_=gates_ps[:, 0:TWOH], func=AF.Sigmoid
            )
            nc.scalar.activation(
                out=acts[:, TWOH : TWOH + H],
                in_=gates_ps[:, TWOH : TWOH + H],
                func=AF.Tanh,
            )
            nc.scalar.activation(
                out=acts[:, TWOH + H : FOURH],
                in_=gates_ps[:, TWOH + H : FOURH],
                func=AF.Sigmoid,
            )

        i_g = acts[:, 0:H]
        f_g = acts[:, H:TWOH]
        g_g = acts[:, TWOH : TWOH + H]
        o_g = acts[:, TWOH + H : FOURH]

        ig_sb = consts.tile([NC, H], f32)
        nc.vector.tensor_tensor(out=ig_sb, in0=i_g, in1=g_g, op=ALU.mult)
        nc.vector.tensor_tensor(out=cnew_sb, in0=f_g, in1=cell_sb, op=ALU.mult)
        nc.vector.tensor_tensor(out=cnew_sb, in0=cnew_sb, in1=ig_sb, op=ALU.add)
        tanhc_sb = consts.tile([NC, H], f32)
        nc.scalar.activation(out=tanhc_sb, in_=cnew_sb, func=AF.Tanh)
        nc.vector.tensor_tensor(out=hnew_sb, in0=o_g, in1=tanhc_sb, op=ALU.mult)

        nc.sync.dma_start(out=cnew_t[:], in_=cnew_sb)
        nc.sync.dma_start(out=hnew_t[:], in_=hnew_sb)

        # h_new^T -> bounce -> AllGather
        hTn_sb = consts.tile([128, 4, NC], f32)
        with tc.tile_pool(name="ps_post", bufs=2, space="PSUM") as ps_post:
            for c in range(4):
                ps = ps_post.tile([128, NC], f32, tag="trh")
                nc.tensor.transpose(
                    out=ps,
                    in_=hnew_sb[:, 128 * c : 128 * (c + 1)],
                    identity=identity[0:NC, 0:NC],
                )
                nc.vector.tensor_copy(out=hTn_sb[:, c, :], in_=ps)
        nc.sync.dma_start(
            out=hT_bounce.ap().rearrange("(c k) n -> k c n", k=128), in_=hTn_sb
        )
        nc.gpsimd.collective_compute(
            "AllGather",
            ALU.bypass,
            replica_groups=[list(range(NCORES))],
            ins=[hT_bounce[:]],
            outs=[hT_all.ap().rearrange("c h n -> (c h) n")],
        )

        # ---------------- output projection (vocab shard) ----------------
        # hT_all [8, 512, 32] -> lhsT tiles [128, cores-in-group, n] per k-chunk
        hA_view = hT_all.ap().rearrange("c (t k) n -> k t c n", k=128)
        CG = 128 // NC  # cores per 128-row n-group
        hA = []
        for g in range(N // 128):
            row = []
            for k in range(H // 128):
                t = hA_pool.tile([128, CG, NC], f32r, tag=f"hA{g}{k}")
                nc.gpsimd.dma_start(
                    out=t, in_=hA_view[:, k, CG * g : CG * (g + 1), :]
                )
                row.append(t)
            hA.append(row)

        bo_bf = consts.tile([1, OC], bf16)
        nc.gpsimd.dma_start(out=bo_bf, in_=bo_t[:])
        ones1_bf = consts.tile([1, 128], bf16)
        nc.vector.memset(ones1_bf, 1.0)

        with tc.tile_pool(name="ps_log", bufs=8, space="PSUM") as ps_log:
            for vh in range(NVH):
                wo_tiles = []
                for k in range(H // 128):
                    t = wo_pool.tile([128, VPH], f32r, tag="wo")
                    nc.gpsimd.dma_start(
                        out=t,
                        in_=wo_t[128 * k : 128 * (k + 1), VPH * vh : VPH * (vh + 1)],
                    )
                    wo_tiles.append(t)
                lps = {}
                for k in range(H // 128):
                    for g in range(N // 128):
                        for v in range(NVT):
                            key = (g, v)
                            if k == 0:
                                lps[key] = ps_log.tile(
                                    [128, VW], f32, tag="lp", name=f"lp{vh}_{g}_{v}"
                                )
                            nc.tensor.matmul(
                                out=lps[key],
                                lhsT=hA[g][k],
                                rhs=wo_tiles[k][:, VW * v : VW * (v + 1)],
                                start=(k == 0),
                                stop=False,
                            )
                for g in range(N // 128):
                    for v in range(NVT):
                        vfull = vh * NVT + v
                        vsl = slice(VW * vfull, VW * (vfull + 1))
                        # bias via K=1 ones matmul, then copy out
                        nc.tensor.matmul(
                            out=lps[(g, v)],
                            lhsT=ones1_bf,
                            rhs=bo_bf[0:1, vsl],
                            start=False,
                            stop=True,
                        )
                        lsb = out_pool.tile([128, VW], f32, tag="lsb")
                        nc.vector.tensor_copy(out=lsb, in_=lps[(g, v)])
                        nc.sync.dma_start(
                            out=logits_t[128 * g : 128 * (g + 1), vsl], in_=lsb
                        )


_NC_CACHE = {}


def _get_program():
    if "nc" not in _NC_CACHE:
        _NC_CACHE["nc"] = build_program()
    return _NC_CACHE["nc"]


def build_in_maps(
    x, encoder_outputs, hidden, cell, emb, W_energy, b_energy,
    W_ih, W_hh, b_ih, b_hh, W_out, b_out,
):
    x = np.asarray(x)
    encoder_outputs = np.ascontiguousarray(np.asarray(encoder_outputs, np.float32))
    hidden = np.asarray(hidden, np.float32)
    cell = np.asarray(cell, np.float32)
    emb = np.ascontiguousarray(np.asarray(emb, np.float32))
    W_energy = np.ascontiguousarray(np.asarray(W_energy, np.float32))
    b_energy = np.asarray(b_energy, np.float32)
    W_ih = np.asarray(W_ih, np.float32)
    W_hh = np.asarray(W_hh, np.float32)
    b_ih = np.asarray(b_ih, np.float32)
    b_hh = np.asarray(b_hh, np.float32)
    W_out = np.asarray(W_out, np.float32)
    b_out = np.asarray(b_out, np.float32)

    # weight layout prep (shared across cores)
    wg_T = np.ascontiguousarray(
        np.concatenate([W_ih, W_hh], axis=1).T
    )  # [1792, 2048]
    be = b_energy.reshape(1, 1)
    bih = b_ih.reshape(1, FOURH)
    bhh = b_hh.reshape(1, FOURH)

    in_maps = []
    for c in range(NCORES):
        nsl = slice(NC * c, NC * (c + 1))
        osl = slice(OC * c, OC * (c + 1))
        in_maps.append(
            {
                "x_idx": np.ascontiguousarray(x[nsl].reshape(NC, 1).astype(np.int32)),
                "enc": np.ascontiguousarray(encoder_outputs[nsl]),
                "h_T": np.ascontiguousarray(hidden[0, nsl, :].T),
                "cell": np.ascontiguousarray(cell[0, nsl, :]),
                "emb": emb,
                "w_energy": W_energy.reshape(1, 3 * H),
                "b_energy": be,
                "w_gates_T": wg_T,
                "b_ih": bih,
                "b_hh": bhh,
                "w_out_T": np.ascontiguousarray(W_out[osl, :].T),
                "b_out": np.ascontiguousarray(b_out[osl].reshape(1, OC)),
            }
        )
    return in_maps


def kernel(**inputs):
    in_maps = build_in_maps(**inputs)
    nc = _get_program()
    trace = bool(int(os.environ.get("KERNEL_TRACE", "0")))
    res = run_bass_kernel_spmd(
        nc, in_maps, core_ids=list(range(NCORES)), trace=trace,
        stitch_traces=trace,
    )
    _NC_CACHE["last_results"] = res

    predictions = np.concatenate(
        [res.results[c]["logits"] for c in range(NCORES)], axis=1
    ).reshape(N, 1, O)
    h_new = np.concatenate(
        [res.results[c]["h_new"] for c in range(NCORES)], axis=0
    )[None]
    c_new = np.concatenate(
        [res.results[c]["c_new"] for c in range(NCORES)], axis=0
    )[None]
    return predictions, h_new, c_new


# revision 21
# speedup vs baseline: 469.1096x; 469.1096x over previous
"""Trainium2 Bass kernel for nn_Decoder (attention decoder step + LSTM cell + vocab head).

Sharding: data-parallel over batch N=256 across 8 cores for the attention+LSTM
part; tensor-parallel over the 32k vocab for the output Linear, with an
on-device AllGather of h_new between the two phases. Host does only slicing /
weight layout transposes and final concatenation.
"""

import os
import sys

sys.path.insert(0, "/opt/trn_rl_repo")

import ml_dtypes
import numpy as np

import concourse.bacc as bacc
import concourse.bass as bass
import concourse.tile as tile
from concourse import mybir
from concourse.bass_utils import run_bass_kernel_spmd
from concourse.masks import make_identity

NCORES = 8
V, E, H, O, S, N = 32000, 256, 512, 32000, 128, 256
NC = N // NCORES          # 32 batch rows per core
OC = O // NCORES          # 4000 vocab entries per core
TWOH = 2 * H              # 1024
FOURH = 4 * H             # 2048
KF = TWOH + E + H         # 1792 gates contraction (context | emb | h)
NKT = KF // 128           # 14 k-tiles
ECH = TWOH // 128         # 8 e-chunks of encoder features
NB = 8                    # batch-group size for softmax/context pipelining
VW = 500                  # vocab tile width for the output matmul
NVH = 2                   # vocab halves (streamed W_out)
VPH = OC // NVH           # 2000 vocab cols per half
NVT = VPH // VW           # 4 vocab tiles per half

f32 = mybir.dt.float32
f32r = mybir.dt.float32r
bf16 = mybir.dt.bfloat16
i32 = mybir.dt.int32
AF = mybir.ActivationFunctionType
ALU = mybir.AluOpType


def _r(ap):
    """View a fp32 AP as float32r for full-rate PE matmuls."""
    return ap.bitcast(f32r)


def build_program(repeats=1, sim_mode=False):
    nc = bacc.Bacc(
        "TRN2", target_bir_lowering=False, debug=False,
        num_devices=1 if sim_mode else NCORES,
    )

    # ---- per-core external I/O ----
    x_t = nc.dram_tensor("x_idx", [NC, 1], i32, kind="ExternalInput")
    enc_t = nc.dram_tensor("enc", [NC, S, TWOH], bf16, kind="ExternalInput")
    hT_t = nc.dram_tensor("h_T", [H, NC], f32, kind="ExternalInput")
    cell_t = nc.dram_tensor("cell", [NC, H], f32, kind="ExternalInput")
    emb_t = nc.dram_tensor("emb", [V, E], f32, kind="ExternalInput")
    we_t = nc.dram_tensor("w_energy", [1, 3 * H], f32, kind="ExternalInput")
    be_t = nc.dram_tensor("b_energy", [1, 1], f32, kind="ExternalInput")
    wg_t = nc.dram_tensor("w_gates_T", [KF, FOURH], bf16, kind="ExternalInput")
    bih_t = nc.dram_tensor("b_ih", [1, FOURH], f32, kind="ExternalInput")
    bhh_t = nc.dram_tensor("b_hh", [1, FOURH], f32, kind="ExternalInput")
    wo_t = nc.dram_tensor("w_out_T", [H, OC], bf16, kind="ExternalInput")
    bo_t = nc.dram_tensor("b_out", [1, OC], f32, kind="ExternalInput")

    logits_t = nc.dram_tensor("logits", [N, OC], f32, kind="ExternalOutput")
    hnew_t = nc.dram_tensor("h_new", [NC, H], f32, kind="ExternalOutput")
    cnew_t = nc.dram_tensor("c_new", [NC, H], f32, kind="ExternalOutput")

    # collective bounce buffers (DRAM)
    hT_bounce = nc.dram_tensor("hT_bounce", [H, NC], f32)
    hT_all = nc.dram_tensor("hT_all", [NCORES, H, NC], f32, addr_space="Shared")

    with tile.TileContext(nc) as tc:
        for rep in range(repeats):
            if rep:
                tc.strict_bb_all_engine_barrier()
            _build_body(
                nc, tc,
                x_t, enc_t, hT_t, cell_t, emb_t, we_t, be_t, wg_t, bih_t, bhh_t,
                wo_t, bo_t, logits_t, hnew_t, cnew_t, hT_bounce, hT_all,
                sim_mode=sim_mode,
            )
    nc.compile()
    return nc


def _build_body(
    nc, tc,
    x_t, enc_t, hT_t, cell_t, emb_t, we_t, be_t, wg_t, bih_t, bhh_t,
    wo_t, bo_t, logits_t, hnew_t, cnew_t, hT_bounce, hT_all,
    sim_mode=False,
):
    from contextlib import ExitStack

    ctx = ExitStack()
    with ctx:
        consts = ctx.enter_context(tc.tile_pool(name="consts", bufs=1))
        enc_pool = ctx.enter_context(tc.tile_pool(name="encp", bufs=1))
        small = ctx.enter_context(tc.tile_pool(name="small", bufs=2))
        prods = ctx.enter_context(tc.tile_pool(name="prods", bufs=2))
        wt_pool = ctx.enter_context(tc.tile_pool(name="wtp", bufs=6))
        wo_pool = ctx.enter_context(tc.tile_pool(name="wop", bufs=2))
        hA_pool = ctx.enter_context(tc.tile_pool(name="hAp", bufs=1))
        out_pool = ctx.enter_context(tc.tile_pool(name="outp", bufs=3))

        # ---------------- constants / small loads ----------------
        identity = consts.tile([128, 128], f32)
        make_identity(nc, identity)

        # W_energy enc part as bf16, replicated to all 128 partitions
        we_e_bf = consts.tile([S, TWOH], bf16)
        nc.gpsimd.dma_start(
            out=we_e_bf, in_=we_t[0:1, H : 3 * H].to_broadcast([S, TWOH])
        )
        # W_energy h part as [128, 4] (k-chunk layout)
        we_h = consts.tile([128, 4], f32r)
        nc.gpsimd.dma_start(
            out=we_h, in_=we_t.ap().rearrange("o (c k) -> (o k) c", k=128)[:, 0:4]
        )
        be_bc = consts.tile([128, 1], f32)
        nc.sync.dma_start(out=be_bc, in_=be_t[0:1, 0:1].to_broadcast([128, 1]))

        # h^T [512, 32] -> SBUF [128, 4, 32]
        hT_sb = consts.tile([128, 4, NC], f32r)
        nc.gpsimd.dma_start(
            out=hT_sb, in_=hT_t.ap().rearrange("(c k) n -> k c n", k=128)
        )
        cell_sb = consts.tile([NC, H], f32)
        nc.sync.dma_start(out=cell_sb, in_=cell_t[:])

        # embedding gather
        x_sb = consts.tile([NC, 1], i32)
        nc.sync.dma_start(out=x_sb, in_=x_t[:])
        emb_sb = consts.tile([NC, E], f32)
        nc.gpsimd.indirect_dma_start(
            out=emb_sb[:],
            out_offset=None,
            in_=emb_t[:],
            in_offset=bass.IndirectOffsetOnAxis(ap=x_sb[:, 0:1], axis=0),
        )

        # enc in SBUF as [s=128, n=32, e=1024] bf16 (cast during DMA)
        enc_bf = enc_pool.tile([S, NC, TWOH], bf16)
        enc_src = enc_t.ap().rearrange("n s e -> s n e")
        DMA_GRP = 4  # n rows per DMA
        for g in range(NC // DMA_GRP):
            sl = slice(g * DMA_GRP, (g + 1) * DMA_GRP)
            nc.sync.dma_start(out=enc_bf[:, sl, :], in_=enc_src[:, sl, :])

        eterm = consts.tile([S, NC], f32)
        ctxT_sb = consts.tile([128, ECH, NC], f32r)
        embT_sb = consts.tile([128, 2, NC], f32r)
        hterm_bc = consts.tile([S, NC], f32)

        with (
            tc.tile_pool(name="ps_attn", bufs=3, space="PSUM") as ps_attn,
            tc.tile_pool(name="ps_ctx", bufs=1, space="PSUM") as ps_ctx,
        ):
            # emb^T chunks [128, 2, 32]
            for c in range(2):
                ps = ps_attn.tile([128, 128], f32, tag="tr")
                nc.tensor.transpose(
                    out=ps[:, 0:NC],
                    in_=emb_sb[:, 128 * c : 128 * (c + 1)],
                    identity=identity[0:NC, 0:NC],
                )
                nc.vector.tensor_copy(out=embT_sb[:, c, :], in_=ps[:, 0:NC])

            # hterm[n] = h[n] . We_h + b_energy, replicated on all partitions
            # via a free-broadcast stationary operand -> out [128, NC]
            hterm_ps = ps_attn.tile([128, 128], f32, tag="tr")
            for c in range(4):
                nc.tensor.matmul(
                    out=hterm_ps[:, 0:NC],
                    lhsT=we_h[:, c : c + 1].to_broadcast([128, 128]),
                    rhs=hT_sb[:, c, :],
                    start=(c == 0),
                    stop=(c == 3),
                )
            nc.vector.tensor_copy(out=hterm_bc, in_=hterm_ps[:, 0:NC])
            nc.vector.tensor_scalar_add(
                out=hterm_bc, in0=hterm_bc, scalar1=be_bc[:, 0:1]
            )

            # ---------------- encoder attention ----------------
            ctx_ps = ps_ctx.tile([128, ECH, NC], f32)

            for bg in range(NC // NB):  # 4 batch groups of NB=8
                nsl = slice(bg * NB, (bg + 1) * NB)
                # energy e-term per n: mul (DVE TT bf16 2x) then reduce via
                # tensor_scalar accumulate (DVE 4x bf16)
                for n in range(bg * NB, (bg + 1) * NB):
                    prod = prods.tile([S, TWOH], bf16, tag="prod")
                    nc.vector.tensor_tensor(
                        out=prod, in0=enc_bf[:, n, :], in1=we_e_bf, op=ALU.mult
                    )
                    dump = prods.tile([S, TWOH], bf16, tag="dump")
                    nc.vector.tensor_scalar(
                        out=dump,
                        in0=prod,
                        scalar1=1.0,
                        scalar2=0.0,
                        op0=ALU.mult,
                        op1=ALU.add,
                        accum_out=eterm[:, n : n + 1],
                    )
                # energy = relu(eterm + hterm) in [s, nb] layout
                en_sn = small.tile([S, NB], f32, tag="en_sn")
                nc.vector.tensor_tensor(
                    out=en_sn,
                    in0=eterm[:, nsl],
                    in1=hterm_bc[:, nsl],
                    op=ALU.add,
                )
                nc.vector.tensor_scalar_max(out=en_sn, in0=en_sn, scalar1=0.0)
                # transpose to [nb, s]
                tr_ps = ps_attn.tile([128, 128], f32, tag="tr")
                nc.tensor.transpose(out=tr_ps[0:NB, :], in_=en_sn, identity=identity)
                en_ns = small.tile([NB, S], f32, tag="en_ns")
                nc.vector.tensor_copy(out=en_ns, in_=tr_ps[0:NB, :])
                # softmax over s
                negmax = small.tile([NB, 1], f32, tag="negmax")
                nc.vector.tensor_reduce(
                    out=negmax, in_=en_ns, axis=mybir.AxisListType.X, op=ALU.max,
                    negate=True,
                )
                att_e = small.tile([NB, S], f32, tag="att_e")
                nc.scalar.activation(
                    out=att_e, in_=en_ns, func=AF.Exp, bias=negmax, scale=1.0
                )
                ssum = small.tile([NB, 1], f32, tag="ssum")
                nc.vector.tensor_reduce(
                    out=ssum, in_=att_e, axis=mybir.AxisListType.X, op=ALU.add
                )
                rinv = small.tile([NB, 1], f32, tag="rinv")
                nc.vector.reciprocal(out=rinv, in_=ssum)
                att = small.tile([NB, S], f32, tag="att")
                nc.vector.tensor_scalar_mul(out=att, in0=att_e, scalar1=rinv)
                # att^T -> [s, nb] bf16
                attT_ps = ps_attn.tile([128, 128], f32, tag="tr")
                nc.tensor.transpose(
                    out=attT_ps[:, 0:NB], in_=att, identity=identity[0:NB, 0:NB]
                )
                attT_bf = small.tile([S, NB], bf16, tag="attT_bf")
                nc.vector.tensor_copy(out=attT_bf, in_=attT_ps[:, 0:NB])
                # context^T chunks: ctx_ps[:, c, n] = enc[n,:,128c:...]^T @ att[n]
                for ni in range(NB):
                    n = bg * NB + ni
                    for c in range(ECH):
                        nc.tensor.matmul(
                            out=ctx_ps[:, c, n : n + 1],
                            lhsT=enc_bf[:, n, 128 * c : 128 * (c + 1)],
                            rhs=attT_bf[:, ni : ni + 1],
                            start=True,
                            stop=True,
                        )

            nc.vector.tensor_copy(out=ctxT_sb, in_=ctx_ps)

        # ---------------- LSTM gates ----------------
        # bias injection k-tile (K=2): row0 = b_ih, row1 = b_hh, ones lhsT
        brow_bf = consts.tile([2, FOURH], bf16)
        nc.gpsimd.dma_start(out=brow_bf[0:1, :], in_=bih_t[:])
        nc.gpsimd.dma_start(out=brow_bf[1:2, :], in_=bhh_t[:])
        ones2_bf = consts.tile([2, NC], bf16)
        nc.vector.memset(ones2_bf, 1.0)

        lhs_tiles = (
            [ctxT_sb[:, c, :] for c in range(ECH)]
            + [embT_sb[:, c, :] for c in range(2)]
            + [hT_sb[:, c, :] for c in range(4)]
        )
        wg_tiles = []
        for ki in range(NKT):
            wk = wt_pool.tile([128, FOURH], f32r, tag="wt")
            nc.gpsimd.dma_start(out=wk, in_=wg_t[128 * ki : 128 * (ki + 1), :])
            wg_tiles.append(wk)

        acts = consts.tile([NC, FOURH], f32)
        hnew_sb = consts.tile([NC, H], f32)
        cnew_sb = consts.tile([NC, H], f32)

        with tc.tile_pool(name="ps_gates", bufs=1, space="PSUM") as ps_gates:
            gates_ps = ps_gates.tile([NC, FOURH], f32)
            for ki in range(NKT):
                for j in range(FOURH // 512):
                    jsl = slice(512 * j, 512 * (j + 1))
                    nc.tensor.matmul(
                        out=gates_ps[:, jsl],
                        lhsT=lhs_tiles[ki],
                        rhs=wg_tiles[ki][:, jsl],
                        start=(ki == 0),
                        stop=False,
                    )
            for j in range(FOURH // 512):
                jsl = slice(512 * j, 512 * (j + 1))
                nc.tensor.matmul(
                    out=gates_ps[:, jsl],
                    lhsT=ones2_bf,
                    rhs=brow_bf[:, jsl],
                    start=False,
                    stop=True,
                )

            # ---------------- LSTM cell ----------------
            nc.scalar.activation(
                out=acts[:, 0:TWOH], in_=gates_ps[:, 0:TWOH], func=AF.Sigmoid
            )
            nc.scalar.activation(
                out=acts[:, TWOH : TWOH + H],
                in_=gates_ps[:, TWOH : TWOH + H],
                func=AF.Tanh,
            )
            nc.scalar.activation(
                out=acts[:, TWOH + H : FOURH],
                in_=gates_ps[:, TWOH + H : FOURH],
                func=AF.Sigmoid,
            )

        i_g = acts[:, 0:H]
        f_g = acts[:, H:TWOH]
        g_g = acts[:, TWOH : TWOH + H]
        o_g = acts[:, TWOH + H : FOURH]

        ig_sb = consts.tile([NC, H], f32)
        nc.vector.tensor_tensor(out=ig_sb, in0=i_g, in1=g_g, op=ALU.mult)
        nc.vector.tensor_tensor(out=cnew_sb, in0=f_g, in1=cell_sb, op=ALU.mult)
        nc.vector.tensor_tensor(out=cnew_sb, in0=cnew_sb, in1=ig_sb, op=ALU.add)
        tanhc_sb = consts.tile([NC, H], f32)
        nc.scalar.activation(out=tanhc_sb, in_=cnew_sb, func=AF.Tanh)
        nc.vector.tensor_tensor(out=hnew_sb, in0=o_g, in1=tanhc_sb, op=ALU.mult)

        nc.sync.dma_start(out=cnew_t[:], in_=cnew_sb)
        nc.sync.dma_start(out=hnew_t[:], in_=hnew_sb)

        # h_new^T -> bounce -> AllGather
        hTn_sb = consts.tile([128, 4, NC], f32)
        with tc.tile_pool(name="ps_post", bufs=2, space="PSUM") as ps_post:
            for c in range(4):
                ps = ps_post.tile([128, NC], f32, tag="trh")
                nc.tensor.transpose(
                    out=ps,
                    in_=hnew_sb[:, 128 * c : 128 * (c + 1)],
                    identity=identity[0:NC, 0:NC],
                )
                nc.vector.tensor_copy(out=hTn_sb[:, c, :], in_=ps)
        nc.sync.dma_start(
            out=hT_bounce.ap().rearrange("(c k) n -> k c n", k=128), in_=hTn_sb
        )
        if sim_mode:
            # timing-only stand-in for the AllGather (TimelineSim can't run
            # collectives); bandwidth-equivalent local copy
            nc.sync.dma_start(out=hT_all[0], in_=hT_bounce[:])
        else:
            nc.gpsimd.collective_compute(
                "AllGather",
                ALU.bypass,
                replica_groups=[list(range(NCORES))],
                ins=[hT_bounce[:]],
                outs=[hT_all.ap().rearrange("c h n -> (c h) n")],
            )

        # ---------------- output projection (vocab shard) ----------------
        # hT_all [8, 512, 32] -> lhsT tiles [128, cores-in-group, n] per k-chunk
        hA_view = hT_all.ap().rearrange("c (t k) n -> k t c n", k=128)
        CG = 128 // NC  # cores per 128-row n-group
        hA = []
        for g in range(N // 128):
            row = []
            for k in range(H // 128):
                t = hA_pool.tile([128, CG, NC], f32r, tag=f"hA{g}{k}")
                nc.gpsimd.dma_start(
                    out=t, in_=hA_view[:, k, CG * g : CG * (g + 1), :]
                )
                row.append(t)
            hA.append(row)

        bo_bf = consts.tile([1, OC], bf16)
        nc.gpsimd.dma_start(out=bo_bf, in_=bo_t[:])
        ones1_bf = consts.tile([1, 128], bf16)
        nc.vector.memset(ones1_bf, 1.0)

        with tc.tile_pool(name="ps_log", bufs=8, space="PSUM") as ps_log:
            for vh in range(NVH):
                wo_tiles = []
                for k in range(H // 128):
                    t = wo_pool.tile([128, VPH], f32r, tag="wo")
                    nc.gpsimd.dma_start(
                        out=t,
                        in_=wo_t[128 * k : 128 * (k + 1), VPH * vh : VPH * (vh + 1)],
                    )
                    wo_tiles.append(t)
                lps = {}
                for k in range(H // 128):
                    for g in range(N // 128):
                        for v in range(NVT):
                            key = (g, v)
                            if k == 0:
                                lps[key] = ps_log.tile(
                                    [128, VW], f32, tag="lp", name=f"lp{vh}_{g}_{v}"
                                )
                            nc.tensor.matmul(
                                out=lps[key],
                                lhsT=hA[g][k],
                                rhs=wo_tiles[k][:, VW * v : VW * (v + 1)],
                                start=(k == 0),
                                stop=False,
                            )
                for g in range(N // 128):
                    for v in range(NVT):
                        vfull = vh * NVT + v
                        vsl = slice(VW * vfull, VW * (vfull + 1))
                        # bias via K=1 ones matmul, then copy out
                        nc.tensor.matmul(
                            out=lps[(g, v)],
                            lhsT=ones1_bf,
                            rhs=bo_bf[0:1, vsl],
                            start=False,
                            stop=True,
                        )
                        lsb = out_pool.tile([128, VW], f32, tag="lsb")
                        if (g * NVT + v) % 2 == 0:
                            nc.vector.tensor_copy(out=lsb, in_=lps[(g, v)])
                        else:
                            nc.scalar.copy(out=lsb, in_=lps[(g, v)])
                        nc.sync.dma_start(
                            out=logits_t[128 * g : 128 * (g + 1), vsl], in_=lsb
                        )


_NC_CACHE = {}


def _get_program():
    if "nc" not in _NC_CACHE:
        _NC_CACHE["nc"] = build_program()
    return _NC_CACHE["nc"]


def build_in_maps(
    x, encoder_outputs, hidden, cell, emb, W_energy, b_energy,
    W_ih, W_hh, b_ih, b_hh, W_out, b_out,
):
    x = np.asarray(x)
    encoder_outputs = np.ascontiguousarray(
        np.asarray(encoder_outputs, np.float32).astype(ml_dtypes.bfloat16)
    )
    hidden = np.asarray(hidden, np.float32)
    cell = np.asarray(cell, np.float32)
    emb = np.ascontiguousarray(np.asarray(emb, np.float32))
    W_energy = np.ascontiguousarray(np.asarray(W_energy, np.float32))
    b_energy = np.asarray(b_energy, np.float32)
    W_ih = np.asarray(W_ih, np.float32)
    W_hh = np.asarray(W_hh, np.float32)
    b_ih = np.asarray(b_ih, np.float32)
    b_hh = np.asarray(b_hh, np.float32)
    W_out = np.asarray(W_out, np.float32)
    b_out = np.asarray(b_out, np.float32)

    # weight layout prep (shared across cores)
    wg_T = np.ascontiguousarray(
        np.concatenate([W_ih, W_hh], axis=1).T.astype(ml_dtypes.bfloat16)
    )  # [1792, 2048] bf16
    be = b_energy.reshape(1, 1)
    bih = b_ih.reshape(1, FOURH)
    bhh = b_hh.reshape(1, FOURH)

    in_maps = []
    for c in range(NCORES):
        nsl = slice(NC * c, NC * (c + 1))
        osl = slice(OC * c, OC * (c + 1))
        in_maps.append(
            {
                "x_idx": np.ascontiguousarray(x[nsl].reshape(NC, 1).astype(np.int32)),
                "enc": np.ascontiguousarray(encoder_outputs[nsl]),
                "h_T": np.ascontiguousarray(hidden[0, nsl, :].T),
                "cell": np.ascontiguousarray(cell[0, nsl, :]),
                "emb": emb,
                "w_energy": W_energy.reshape(1, 3 * H),
                "b_energy": be,
                "w_gates_T": wg_T,
                "b_ih": bih,
                "b_hh": bhh,
                "w_out_T": np.ascontiguousarray(W_out[osl, :].T.astype(ml_dtypes.bfloat16)),
                "b_out": np.ascontiguousarray(b_out[osl].reshape(1, OC)),
            }
        )
    return in_maps


def kernel(**inputs):
    in_maps = build_in_maps(**inputs)
    nc = _get_program()
    trace = bool(int(os.environ.get("KERNEL_TRACE", "0")))
    res = run_bass_kernel_spmd(
        nc, in_maps, core_ids=list(range(NCORES)), trace=trace,
        stitch_traces=trace,
    )
    _NC_CACHE["last_results"] = res

    predictions = np.concatenate(
        [res.results[c]["logits"] for c in range(NCORES)], axis=1
    ).reshape(N, 1, O)
    h_new = np.concatenate(
        [res.results[c]["h_new"] for c in range(NCORES)], axis=0
    )[None]
    c_new = np.concatenate(
        [res.results[c]["c_new"] for c in range(NCORES)], axis=0
    )[None]
    return predictions, h_new, c_new
